# revision 56
# baseline (speedup 1.0000x reference)
"""Trainium2 Bass kernel for the BiauralProcessor problem.

Strategy (per core; batch data-parallel, 2 batches/core on 8 cores):
  - Host packs inputs transposed (freq on partitions) and bf16:
      packed[0:64]   = [zeros32 | L_b0 | zeros32 | R_b0 | ones]
      packed[64:128] = [zeros32 | L_b1 | zeros32 | R_b1 | ones]
  - ITD: for each 128-time window, a PE band matmul with stationary
    R-window [64,128] and streamed L [64,192] produces the full lag
    band in PSUM; every lag in [-32,32] lives on a diagonal.  After a
    DVE/ACT eviction to SBUF (bf16), one DMA with a fused
    (row_pitch+1) element step extracts a contiguous 65-wide diagonal
    slab [128,65] straight to DRAM.  The host selects the 16 delay
    columns.
  - ILD: per-window N=1 matmuls against a ones column accumulate the
    freq-sums L_s/R_s into PSUM; at the end the DVE computes
    (L_s-R_s)/(L_s+R_s+1e-6) and the ACT engine applies the gaussian
    tuning (Square then Exp) for the 8 preferences.

ISA constraint that shaped the code: every engine instruction carries
at most ONE sync-wait (waits on the same semaphore merge).  Hence:
  - two input DMAs only (L region incl. pads / R region incl. ones),
  - the band SBUF staging area is one persistent buffer (no pool slot
    recycling -> evictions wait only on PE, diag DMAs only on the
    evictor),
  - psum slot reuse is preceded by a dummy 1x1 LDWEIGHTS reading the
    last evicted tile, so the PE observes the evictor's semaphore and
    the reusing matmul keeps a single (PE) wait,
  - activation biases are float immediates (static const tensors).
"""

import numpy as np

import concourse.bass as bass
import concourse.bacc as bacc
import concourse.mybir as mybir
import concourse.tile as tile
from concourse.bass import _add_dep_helper
from concourse.bass_utils import run_bass_kernel_spmd

# ---- problem constants (hardcoded from the spec) ----
B, T, F = 16, 16000, 64
N_CORES = 8
B_LOC = B // N_CORES            # 2 batches per core
MAX_ITD, N_ITD, N_ILD = 32, 16, 8
ITD_DELAYS = np.round(np.linspace(-MAX_ITD, MAX_ITD, N_ITD)).astype(np.int64)
ILD_PREFS = np.linspace(-1.0, 1.0, N_ILD).astype(np.float32)

PAD = 32                        # zero halo around L for the band stream
WIN = 128                       # output times per window
NSTREAM = 192                   # streamed L columns per window
SLABW = 65                      # extracted diagonals (lags -32..32)
GROUP = 125                     # windows per ILD group (one group: the
                                # sums psum tiles are never recycled)

F32 = mybir.dt.float32
BF16 = mybir.dt.bfloat16


def build_nc(t_len=T, group=GROUP):
    nw = t_len // WIN           # windows per batch
    ngroups = nw // group
    assert nw % group == 0
    wtot = PAD + t_len + PAD + t_len + 1   # trailing ones column
    l_off = PAD
    r_off = PAD + t_len + PAD
    ones_off = wtot - 1

    n_pairs = (nw + 1) // 2     # band tiles per batch (last may be single)

    nc = bacc.Bacc("TRN2", target_bir_lowering=False, debug=False)
    packed = nc.dram_tensor("packed", [128, wtot], BF16, kind="ExternalInput")
    # band pairs are staged in SBUF and shipped 8 pairs per DMA (the SP
    # sequencer costs ~600ns per DMA instruction - keep the count low)
    GB = 8                       # pairs per band DMA
    n_g8 = (n_pairs + GB - 1) // GB
    band_out = nc.dram_tensor(
        "band", [B_LOC, n_g8, 128, GB * 2 * NSTREAM], BF16, kind="ExternalOutput")
    # raw [m, w*8+k] layout (contiguous dump; host rearranges to [t, k])
    ild = nc.dram_tensor(
        "ild", [B_LOC, ngroups, 128, group * N_ILD], F32, kind="ExternalOutput")

    # gaussian-tuning bias constants, initialized in the preamble (before
    # the Tile region, behind an all-engine barrier) so activations using
    # them carry no runtime dependency.
    bias_aps = []
    for k in range(N_ILD):
        t = nc.alloc_sbuf_tensor(f"const-bias-{k}", [128, 1], F32)
        nc.gpsimd.memset(t.ap(), float(-ILD_PREFS[k] / 0.3))
        bias_aps.append(t.ap())
    nc.all_engine_barrier()

    with tile.TileContext(nc) as tc:
        with (
            tc.tile_pool(name="inp", bufs=1) as inp_pool,
            tc.tile_pool(name="band_sb", bufs=3) as band_pool,
            tc.tile_pool(name="sums_sb", bufs=2) as sums_sb_pool,
            tc.tile_pool(name="ild_work", bufs=2) as work_pool,
            tc.tile_pool(name="out8", bufs=2) as out8_pool,
            tc.tile_pool(name="band_ps", bufs=5, space=bass.MemorySpace.PSUM) as band_ps_pool,
            tc.tile_pool(name="sums_ps", bufs=2, space=bass.MemorySpace.PSUM) as sums_ps_pool,
        ):
            lr = inp_pool.tile([128, wtot], BF16)
            # input loads go on the ACT HWDGE ring (nc.scalar) so the output
            # DMAs on the sync ring are not stuck behind them (FIFO per
            # ring); chunks interleave L/R so early windows start ASAP
            nsplit = 4
            chunks = []
            for ci in range(nsplit):
                for r0, r1 in ((0, r_off), (r_off, wtot)):
                    csz = (r1 - r0 + nsplit - 1) // nsplit
                    c0 = r0 + ci * csz
                    c1 = min(r0 + (ci + 1) * csz, r1)
                    chunks.append((c0, c1))
            for c0, c1 in chunks:
                nc.scalar.dma_start(lr[:, c0:c1], packed[:, c0:c1])
            ones = lr[:, ones_off:ones_off + 1]

            evicted = []          # evicted band slices, in order
            staging = {}          # (b, g8) -> staging tile

            for g in range(ngroups):
                sums_ps = [
                    sums_ps_pool.tile([128, 2 * group], F32, tag="sums",
                                      name=f"sums_g{g}b{bb}")
                    for bb in range(B_LOC)
                ]
                wl = 0
                while wl < group:
                    npair = 2 if wl + 1 < group else 1
                    w0 = g * group + wl
                    for b in range(B_LOC):
                        bp = 64 * b
                        ps = band_ps_pool.tile([128, NSTREAM * npair], F32, tag="band")
                        if len(evicted) >= 5:      # band psum pool bufs
                            prev = evicted[len(evicted) - 5]
                            nc.tensor.ldweights(prev[0:1, 0:1])
                        for i in range(npair):
                            tr = (w0 + i) * WIN
                            bi_band = nc.tensor.matmul(
                                ps[:, i * NSTREAM:(i + 1) * NSTREAM],
                                lr[bp:bp + 64, r_off + tr: r_off + tr + WIN],
                                lr[bp:bp + 64, l_off + tr - PAD: l_off + tr - PAD + NSTREAM],
                            )
                            # freq-sum matmuls for ILD (N=1, ones rhs);
                            # R-sum first: same stationary as the band matmul.
                            # Ordering edges keep the scheduler from hoisting
                            # all the ready N=1 sums matmuls into one burst
                            # that starves the eviction engines.
                            w_abs = wl + i
                            bi_rs = nc.tensor.matmul(
                                sums_ps[b][:, 2 * w_abs + 1: 2 * w_abs + 2],
                                lr[bp:bp + 64, r_off + tr: r_off + tr + WIN],
                                ones[bp:bp + 64, :],
                            )
                            bi_ls = nc.tensor.matmul(
                                sums_ps[b][:, 2 * w_abs: 2 * w_abs + 1],
                                lr[bp:bp + 64, l_off + tr: l_off + tr + WIN],
                                ones[bp:bp + 64, :],
                            )
                            _add_dep_helper(bi_rs.ins, bi_band.ins, sync=False,
                                            reason="keep sums after band")
                            _add_dep_helper(bi_ls.ins, bi_band.ins, sync=False,
                                            reason="keep sums after band")
                        width = NSTREAM * npair
                        p_idx = w0 // 2
                        g8, pg = p_idx // GB, p_idx % GB
                        if (b, g8) not in staging:
                            staging[(b, g8)] = band_pool.tile(
                                [128, GB * 2 * NSTREAM], BF16, tag="band",
                                name=f"stage_b{b}g{g8}")
                        stage = staging[(b, g8)]
                        band = stage[:, pg * 2 * NSTREAM: pg * 2 * NSTREAM + width]
                        if (p_idx + b) % 2 == 0:
                            nc.vector.tensor_copy(band, ps[:])
                        else:
                            nc.scalar.copy(band, ps[:])
                        evicted.append(band)
                        # group filled (or last pair): ship 8 pairs in one DMA
                        if pg == GB - 1 or p_idx == n_pairs - 1:
                            gw = pg * 2 * NSTREAM + width
                            nc.sync.dma_start(band_out[b, g8, :, :gw],
                                              stage[:, :gw])
                    wl += npair

                # ---- ILD for this group ----
                for b in range(B_LOC):
                    sums_sb = sums_sb_pool.tile([128, 2 * group], F32)
                    nc.vector.tensor_copy(sums_sb[:], sums_ps[b][:])
                    sap = sums_sb[:]
                    ls = bass.AP(sap.tensor, sap.offset,
                                 [[2 * group, 128], [2, group]])
                    rs = bass.AP(sap.tensor, sap.offset + 1,
                                 [[2 * group, 128], [2, group]])
                    d_t = work_pool.tile([128, group], F32, tag="d")
                    nc.vector.tensor_sub(d_t[:], ls, rs)
                    s_t = work_pool.tile([128, group], F32, tag="s")
                    nc.vector.scalar_tensor_tensor(
                        s_t[:], ls, 1e-6, rs,
                        op0=mybir.AluOpType.add, op1=mybir.AluOpType.add,
                    )
                    r_t = work_pool.tile([128, group], F32, tag="r")
                    nc.vector.reciprocal(r_t[:], s_t[:])
                    i_t = work_pool.tile([128, group], F32, tag="i")
                    nc.vector.tensor_mul(i_t[:], d_t[:], r_t[:])
                    o8 = out8_pool.tile([128, group * N_ILD], F32)
                    o8ap = o8[:]
                    for k in range(N_ILD):
                        q_t = work_pool.tile([128, group], F32, tag="q")
                        nc.scalar.activation(
                            q_t[:], i_t[:],
                            mybir.ActivationFunctionType.Square,
                            bias=bias_aps[k], scale=float(1.0 / 0.3),
                        )
                        o_view = bass.AP(
                            o8ap.tensor, o8ap.offset + k,
                            [[group * N_ILD, 128], [N_ILD, group]],
                        )
                        nc.scalar.activation(
                            o_view, q_t[:],
                            mybir.ActivationFunctionType.Exp,
                            bias=0.0, scale=-0.5,
                        )
                    nc.sync.dma_start(ild[b, g], o8[:])
    nc.compile()
    return nc


_CACHE = {}


def _get_nc():
    if "nc" not in _CACHE:
        _CACHE["nc"] = build_nc()
    return _CACHE["nc"]


def _pack_inputs(left, right, t_len=T):
    """-> list of per-core input maps. left/right: [B, T, F] f32."""
    import ml_dtypes
    wtot = PAD + t_len + PAD + t_len + 1
    lt = np.ascontiguousarray(left.transpose(0, 2, 1)).astype(ml_dtypes.bfloat16)
    rt = np.ascontiguousarray(right.transpose(0, 2, 1)).astype(ml_dtypes.bfloat16)
    in_maps = []
    for c in range(N_CORES):
        pk = np.zeros((128, wtot), dtype=ml_dtypes.bfloat16)
        pk[:, wtot - 1] = 1.0
        for i in range(B_LOC):
            bidx = B_LOC * c + i
            rows = slice(64 * i, 64 * i + 64)
            pk[rows, PAD:PAD + t_len] = lt[bidx]
            pk[rows, PAD + t_len + PAD: wtot - 1] = rt[bidx]
        in_maps.append({"packed": pk})
    return in_maps


def _band_to_slab(band_f32, t_len):
    """[n_g8, 128, GB*2*NSTREAM] f32 -> slab [t_len, SLABW]:
    slab[128*(2*(GB*g+pg)+i)+m, j] = band[g, m, (pg*2+i)*NSTREAM + m + j]."""
    n_g8, p128, gw = band_f32.shape
    nw = t_len // WIN
    n_win_slots = n_g8 * gw // NSTREAM
    a4 = band_f32.reshape(n_g8, p128, gw // NSTREAM, NSTREAM)
    s0, s1, s2, s3 = a4.strides
    sv = np.lib.stride_tricks.as_strided(
        a4, shape=(n_g8, gw // NSTREAM, WIN, SLABW), strides=(s0, s2, s1 + s3, s3))
    return sv.reshape(n_win_slots, WIN, SLABW)[:nw].reshape(nw * WIN, SLABW)


def _unpack_outputs(results, t_len=T):
    itd = np.zeros((B, t_len, N_ITD), dtype=np.float32)
    ild = np.zeros((B, t_len, N_ILD), dtype=np.float32)
    for c in range(N_CORES):
        band_c = np.asarray(results[c]["band"]).astype(np.float32)
        ild_c = np.asarray(results[c]["ild"])  # [B_LOC, ngroups, 128, group*8]
        ng = ild_c.shape[1]
        grp = ild_c.shape[3] // N_ILD
        for i in range(B_LOC):
            bidx = B_LOC * c + i
            # [g, m, w, k] -> t = 128*(g*grp + w) + m
            ild[bidx] = (
                ild_c[i]
                .reshape(ng, 128, grp, N_ILD)
                .transpose(0, 2, 1, 3)
                .reshape(t_len, N_ILD)
            )
            slab = _band_to_slab(band_c[i], t_len)   # [T, 65]
            for k, d in enumerate(ITD_DELAYS):
                d = int(d)
                j = MAX_ITD - d
                if d >= 0:
                    itd[bidx, :, k] = slab[:, j]
                else:
                    itd[bidx, -d:, k] = slab[: t_len + d, j]
    return itd, ild


def kernel(left_spikes, right_spikes):
    left = np.asarray(left_spikes, dtype=np.float32)
    right = np.asarray(right_spikes, dtype=np.float32)
    nc = _get_nc()
    in_maps = _pack_inputs(left, right)
    res = run_bass_kernel_spmd(nc, in_maps, list(range(N_CORES)))
    _CACHE["last_result"] = res
    return _unpack_outputs(res.results)


# revision 61
# speedup vs baseline: 1.1122x; 1.1122x over previous
"""Trainium2 Bass kernel for the BiauralProcessor problem.

Strategy (per core; batch data-parallel, 2 batches/core on 8 cores):
  - Host packs inputs transposed (freq on partitions) and bf16:
      packed[0:64]   = [zeros32 | L_b0 | zeros32 | R_b0 | ones]
      packed[64:128] = [zeros32 | L_b1 | zeros32 | R_b1 | ones]
  - ITD: for each 128-time window, a PE band matmul with stationary
    R-window [64,128] and streamed L [64,192] produces the full lag
    band in PSUM; every lag in [-32,32] lives on a diagonal.  After a
    DVE/ACT eviction to SBUF (bf16), one DMA with a fused
    (row_pitch+1) element step extracts a contiguous 65-wide diagonal
    slab [128,65] straight to DRAM.  The host selects the 16 delay
    columns.
  - ILD: per-window N=1 matmuls against a ones column accumulate the
    freq-sums L_s/R_s into PSUM; at the end the DVE computes
    (L_s-R_s)/(L_s+R_s+1e-6) and the ACT engine applies the gaussian
    tuning (Square then Exp) for the 8 preferences.

ISA constraint that shaped the code: every engine instruction carries
at most ONE sync-wait (waits on the same semaphore merge).  Hence:
  - two input DMAs only (L region incl. pads / R region incl. ones),
  - the band SBUF staging area is one persistent buffer (no pool slot
    recycling -> evictions wait only on PE, diag DMAs only on the
    evictor),
  - psum slot reuse is preceded by a dummy 1x1 LDWEIGHTS reading the
    last evicted tile, so the PE observes the evictor's semaphore and
    the reusing matmul keeps a single (PE) wait,
  - activation biases are float immediates (static const tensors).
"""

import numpy as np

import concourse.bass as bass
import concourse.bacc as bacc
import concourse.mybir as mybir
import concourse.tile as tile
from concourse.bass_utils import run_bass_kernel_spmd

# ---- problem constants (hardcoded from the spec) ----
B, T, F = 16, 16000, 64
N_CORES = 8
B_LOC = B // N_CORES            # 2 batches per core
MAX_ITD, N_ITD, N_ILD = 32, 16, 8
ITD_DELAYS = np.round(np.linspace(-MAX_ITD, MAX_ITD, N_ITD)).astype(np.int64)
ILD_PREFS = np.linspace(-1.0, 1.0, N_ILD).astype(np.float32)

PAD = 32                        # zero halo around L for the band stream
WIN = 128                       # output times per window
NSTREAM = 192                   # streamed L columns per window
SLABW = 65                      # extracted diagonals (lags -32..32)
GROUP = 125                     # windows per ILD group (one group: the
                                # sums psum tiles are never recycled)

F32 = mybir.dt.float32
BF16 = mybir.dt.bfloat16


def build_nc(t_len=T, group=GROUP):
    nw = t_len // WIN           # windows per batch
    ngroups = nw // group
    assert nw % group == 0
    wtot = PAD + t_len + PAD + t_len + 1   # trailing ones column
    l_off = PAD
    r_off = PAD + t_len + PAD
    ones_off = wtot - 1

    n_pairs = (nw + 1) // 2     # band tiles per batch (last may be single)

    nc = bacc.Bacc("TRN2", target_bir_lowering=False, debug=False)
    packed = nc.dram_tensor("packed", [128, wtot], BF16, kind="ExternalInput")
    # band pairs are staged in SBUF and shipped 8 pairs per DMA (the SP
    # sequencer costs ~600ns per DMA instruction - keep the count low)
    GB = 8                       # pairs per band DMA
    n_g8 = (n_pairs + GB - 1) // GB
    band_out = nc.dram_tensor(
        "band", [B_LOC, n_g8, 128, GB * 2 * NSTREAM], BF16, kind="ExternalOutput")
    # raw [m, w*8+k] layout (contiguous dump; host rearranges to [t, k])
    ild = nc.dram_tensor(
        "ild", [B_LOC, ngroups, 128, group * N_ILD], F32, kind="ExternalOutput")

    # gaussian-tuning bias constants, initialized in the preamble (before
    # the Tile region, behind an all-engine barrier) so activations using
    # them carry no runtime dependency.
    bias_aps = []
    for k in range(N_ILD):
        t = nc.alloc_sbuf_tensor(f"const-bias-{k}", [128, 1], F32)
        nc.gpsimd.memset(t.ap(), float(-ILD_PREFS[k] / 0.3))
        bias_aps.append(t.ap())
    nc.all_engine_barrier()

    with tile.TileContext(nc) as tc:
        with (
            tc.tile_pool(name="inp", bufs=1) as inp_pool,
            tc.tile_pool(name="band_sb", bufs=3) as band_pool,
            tc.tile_pool(name="sums_sb", bufs=2) as sums_sb_pool,
            tc.tile_pool(name="ild_work", bufs=2) as work_pool,
            tc.tile_pool(name="out8", bufs=2) as out8_pool,
            tc.tile_pool(name="band_ps", bufs=5, space=bass.MemorySpace.PSUM) as band_ps_pool,
            tc.tile_pool(name="sums_ps", bufs=2, space=bass.MemorySpace.PSUM) as sums_ps_pool,
        ):
            lr = inp_pool.tile([128, wtot], BF16)
            # input loads go on the ACT HWDGE ring (nc.scalar) so the output
            # DMAs on the sync ring are not stuck behind them (FIFO per
            # ring); chunks interleave L/R so early windows start ASAP
            nsplit = 4
            chunks = []
            for ci in range(nsplit):
                for r0, r1 in ((0, r_off), (r_off, wtot)):
                    csz = (r1 - r0 + nsplit - 1) // nsplit
                    c0 = r0 + ci * csz
                    c1 = min(r0 + (ci + 1) * csz, r1)
                    chunks.append((c0, c1))
            for c0, c1 in chunks:
                nc.scalar.dma_start(lr[:, c0:c1], packed[:, c0:c1])
            ones = lr[:, ones_off:ones_off + 1]

            evicted = []          # evicted band slices, in order
            staging = {}          # (b, g8) -> staging tile

            for g in range(ngroups):
                sums_ps = [
                    sums_ps_pool.tile([128, 2 * group], F32, tag="sums",
                                      name=f"sums_g{g}b{bb}")
                    for bb in range(B_LOC)
                ]
                wl = 0
                while wl < group:
                    npair = 2 if wl + 1 < group else 1
                    w0 = g * group + wl
                    for b in range(B_LOC):
                        bp = 64 * b
                        ps = band_ps_pool.tile([128, NSTREAM * npair], F32, tag="band")
                        for i in range(npair):
                            tr = (w0 + i) * WIN
                            nc.tensor.matmul(
                                ps[:, i * NSTREAM:(i + 1) * NSTREAM],
                                lr[bp:bp + 64, r_off + tr: r_off + tr + WIN],
                                lr[bp:bp + 64, l_off + tr - PAD: l_off + tr - PAD + NSTREAM],
                            )
                            # freq-sum matmuls for ILD (N=1, ones rhs);
                            # R-sum first: same stationary as the band matmul
                            w_abs = wl + i
                            nc.tensor.matmul(
                                sums_ps[b][:, 2 * w_abs + 1: 2 * w_abs + 2],
                                lr[bp:bp + 64, r_off + tr: r_off + tr + WIN],
                                ones[bp:bp + 64, :],
                            )
                            nc.tensor.matmul(
                                sums_ps[b][:, 2 * w_abs: 2 * w_abs + 1],
                                lr[bp:bp + 64, l_off + tr: l_off + tr + WIN],
                                ones[bp:bp + 64, :],
                            )
                        width = NSTREAM * npair
                        p_idx = w0 // 2
                        g8, pg = p_idx // GB, p_idx % GB
                        if (b, g8) not in staging:
                            staging[(b, g8)] = band_pool.tile(
                                [128, GB * 2 * NSTREAM], BF16, tag="band",
                                name=f"stage_b{b}g{g8}")
                        stage = staging[(b, g8)]
                        band = stage[:, pg * 2 * NSTREAM: pg * 2 * NSTREAM + width]
                        if (p_idx + b) % 2 == 0:
                            nc.vector.tensor_copy(band, ps[:])
                        else:
                            nc.scalar.copy(band, ps[:])
                        evicted.append(band)
                        # group filled (or last pair): ship 8 pairs in one DMA
                        if pg == GB - 1 or p_idx == n_pairs - 1:
                            gw = pg * 2 * NSTREAM + width
                            nc.sync.dma_start(band_out[b, g8, :, :gw],
                                              stage[:, :gw])
                    wl += npair

                # ---- ILD for this group ----
                for b in range(B_LOC):
                    sums_sb = sums_sb_pool.tile([128, 2 * group], F32)
                    nc.vector.tensor_copy(sums_sb[:], sums_ps[b][:])
                    sap = sums_sb[:]
                    ls = bass.AP(sap.tensor, sap.offset,
                                 [[2 * group, 128], [2, group]])
                    rs = bass.AP(sap.tensor, sap.offset + 1,
                                 [[2 * group, 128], [2, group]])
                    d_t = work_pool.tile([128, group], F32, tag="d")
                    nc.vector.tensor_sub(d_t[:], ls, rs)
                    s_t = work_pool.tile([128, group], F32, tag="s")
                    nc.vector.scalar_tensor_tensor(
                        s_t[:], ls, 1e-6, rs,
                        op0=mybir.AluOpType.add, op1=mybir.AluOpType.add,
                    )
                    r_t = work_pool.tile([128, group], F32, tag="r")
                    nc.vector.reciprocal(r_t[:], s_t[:])
                    i_t = work_pool.tile([128, group], F32, tag="i")
                    nc.vector.tensor_mul(i_t[:], d_t[:], r_t[:])
                    o8 = out8_pool.tile([128, group * N_ILD], F32)
                    o8ap = o8[:]
                    for k in range(N_ILD):
                        q_t = work_pool.tile([128, group], F32, tag="q")
                        nc.scalar.activation(
                            q_t[:], i_t[:],
                            mybir.ActivationFunctionType.Square,
                            bias=bias_aps[k], scale=float(1.0 / 0.3),
                        )
                        o_view = bass.AP(
                            o8ap.tensor, o8ap.offset + k,
                            [[group * N_ILD, 128], [N_ILD, group]],
                        )
                        nc.scalar.activation(
                            o_view, q_t[:],
                            mybir.ActivationFunctionType.Exp,
                            bias=0.0, scale=-0.5,
                        )
                    nc.sync.dma_start(ild[b, g], o8[:])
    nc.compile()
    return nc


_CACHE = {}


def _get_nc():
    if "nc" not in _CACHE:
        _CACHE["nc"] = build_nc()
    return _CACHE["nc"]


def _pack_inputs(left, right, t_len=T):
    """-> list of per-core input maps. left/right: [B, T, F] f32."""
    import ml_dtypes
    wtot = PAD + t_len + PAD + t_len + 1
    lt = np.ascontiguousarray(left.transpose(0, 2, 1)).astype(ml_dtypes.bfloat16)
    rt = np.ascontiguousarray(right.transpose(0, 2, 1)).astype(ml_dtypes.bfloat16)
    in_maps = []
    for c in range(N_CORES):
        pk = np.zeros((128, wtot), dtype=ml_dtypes.bfloat16)
        pk[:, wtot - 1] = 1.0
        for i in range(B_LOC):
            bidx = B_LOC * c + i
            rows = slice(64 * i, 64 * i + 64)
            pk[rows, PAD:PAD + t_len] = lt[bidx]
            pk[rows, PAD + t_len + PAD: wtot - 1] = rt[bidx]
        in_maps.append({"packed": pk})
    return in_maps


def _band_to_slab(band_f32, t_len):
    """[n_g8, 128, GB*2*NSTREAM] f32 -> slab [t_len, SLABW]:
    slab[128*(2*(GB*g+pg)+i)+m, j] = band[g, m, (pg*2+i)*NSTREAM + m + j]."""
    n_g8, p128, gw = band_f32.shape
    nw = t_len // WIN
    n_win_slots = n_g8 * gw // NSTREAM
    a4 = band_f32.reshape(n_g8, p128, gw // NSTREAM, NSTREAM)
    s0, s1, s2, s3 = a4.strides
    sv = np.lib.stride_tricks.as_strided(
        a4, shape=(n_g8, gw // NSTREAM, WIN, SLABW), strides=(s0, s2, s1 + s3, s3))
    return sv.reshape(n_win_slots, WIN, SLABW)[:nw].reshape(nw * WIN, SLABW)


def _unpack_outputs(results, t_len=T):
    itd = np.zeros((B, t_len, N_ITD), dtype=np.float32)
    ild = np.zeros((B, t_len, N_ILD), dtype=np.float32)
    for c in range(N_CORES):
        band_c = np.asarray(results[c]["band"]).astype(np.float32)
        ild_c = np.asarray(results[c]["ild"])  # [B_LOC, ngroups, 128, group*8]
        ng = ild_c.shape[1]
        grp = ild_c.shape[3] // N_ILD
        for i in range(B_LOC):
            bidx = B_LOC * c + i
            # [g, m, w, k] -> t = 128*(g*grp + w) + m
            ild[bidx] = (
                ild_c[i]
                .reshape(ng, 128, grp, N_ILD)
                .transpose(0, 2, 1, 3)
                .reshape(t_len, N_ILD)
            )
            slab = _band_to_slab(band_c[i], t_len)   # [T, 65]
            for k, d in enumerate(ITD_DELAYS):
                d = int(d)
                j = MAX_ITD - d
                if d >= 0:
                    itd[bidx, :, k] = slab[:, j]
                else:
                    itd[bidx, -d:, k] = slab[: t_len + d, j]
    return itd, ild


def kernel(left_spikes, right_spikes):
    left = np.asarray(left_spikes, dtype=np.float32)
    right = np.asarray(right_spikes, dtype=np.float32)
    nc = _get_nc()
    in_maps = _pack_inputs(left, right)
    res = run_bass_kernel_spmd(nc, in_maps, list(range(N_CORES)))
    _CACHE["last_result"] = res
    return _unpack_outputs(res.results)


# revision 62
# speedup vs baseline: 1.1696x; 1.0516x over previous
"""Trainium2 Bass kernel for the BiauralProcessor problem.

Strategy (per core; batch data-parallel, 2 batches/core on 8 cores):
  - Host packs inputs transposed (freq on partitions) and bf16:
      packed[0:64]   = [zeros32 | L_b0 | zeros32 | R_b0 | ones]
      packed[64:128] = [zeros32 | L_b1 | zeros32 | R_b1 | ones]
  - ITD: for each 128-time window, a PE band matmul with stationary
    R-window [64,128] and streamed L [64,192] produces the full lag
    band in PSUM; every lag in [-32,32] lives on a diagonal.  After a
    DVE/ACT eviction to SBUF (bf16), one DMA with a fused
    (row_pitch+1) element step extracts a contiguous 65-wide diagonal
    slab [128,65] straight to DRAM.  The host selects the 16 delay
    columns.
  - ILD: per-window N=1 matmuls against a ones column accumulate the
    freq-sums L_s/R_s into PSUM; at the end the DVE computes
    (L_s-R_s)/(L_s+R_s+1e-6) and the ACT engine applies the gaussian
    tuning (Square then Exp) for the 8 preferences.

ISA constraint that shaped the code: every engine instruction carries
at most ONE sync-wait (waits on the same semaphore merge).  Hence:
  - two input DMAs only (L region incl. pads / R region incl. ones),
  - the band SBUF staging area is one persistent buffer (no pool slot
    recycling -> evictions wait only on PE, diag DMAs only on the
    evictor),
  - psum slot reuse is preceded by a dummy 1x1 LDWEIGHTS reading the
    last evicted tile, so the PE observes the evictor's semaphore and
    the reusing matmul keeps a single (PE) wait,
  - activation biases are float immediates (static const tensors).
"""

import numpy as np

import concourse.bass as bass
import concourse.bacc as bacc
import concourse.mybir as mybir
import concourse.tile as tile
from concourse.bass_utils import run_bass_kernel_spmd

# ---- problem constants (hardcoded from the spec) ----
B, T, F = 16, 16000, 64
N_CORES = 8
B_LOC = B // N_CORES            # 2 batches per core
MAX_ITD, N_ITD, N_ILD = 32, 16, 8
ITD_DELAYS = np.round(np.linspace(-MAX_ITD, MAX_ITD, N_ITD)).astype(np.int64)
ILD_PREFS = np.linspace(-1.0, 1.0, N_ILD).astype(np.float32)

PAD = 32                        # zero halo around L for the band stream
WIN = 128                       # output times per window
NSTREAM = 192                   # streamed L columns per window
SLABW = 65                      # extracted diagonals (lags -32..32)
GROUP = 125                     # windows per ILD group (one group: the
                                # sums psum tiles are never recycled)

F32 = mybir.dt.float32
BF16 = mybir.dt.bfloat16


def build_nc(t_len=T, group=GROUP):
    nw = t_len // WIN           # windows per batch
    ngroups = nw // group
    assert nw % group == 0
    wtot = PAD + t_len + PAD + t_len + 1   # trailing ones column
    l_off = PAD
    r_off = PAD + t_len + PAD
    ones_off = wtot - 1

    n_pairs = (nw + 1) // 2     # band tiles per batch (last may be single)

    nc = bacc.Bacc("TRN2", target_bir_lowering=False, debug=False)
    packed = nc.dram_tensor("packed", [128, wtot], BF16, kind="ExternalInput")
    # band pairs are staged in SBUF and shipped 8 pairs per DMA (the SP
    # sequencer costs ~600ns per DMA instruction - keep the count low)
    GB = 8                       # pairs per band DMA
    n_g8 = (n_pairs + GB - 1) // GB
    band_out = nc.dram_tensor(
        "band", [B_LOC, n_g8, 128, GB * 2 * NSTREAM], BF16, kind="ExternalOutput")
    # raw [m, w*8+k] layout (contiguous dump; host rearranges to [t, k])
    ild = nc.dram_tensor(
        "ild", [B_LOC, ngroups, 128, group * N_ILD], F32, kind="ExternalOutput")

    # gaussian-tuning bias constants, initialized in the preamble (before
    # the Tile region, behind an all-engine barrier) so activations using
    # them carry no runtime dependency.
    bias_aps = []
    for k in range(N_ILD):
        t = nc.alloc_sbuf_tensor(f"const-bias-{k}", [128, 1], F32)
        nc.gpsimd.memset(t.ap(), float(-ILD_PREFS[k] / 0.3))
        bias_aps.append(t.ap())
    nc.all_engine_barrier()

    with tile.TileContext(nc) as tc:
        with (
            tc.tile_pool(name="inp", bufs=1) as inp_pool,
            tc.tile_pool(name="band_sb", bufs=3) as band_pool,
            tc.tile_pool(name="sums_sb", bufs=2) as sums_sb_pool,
            tc.tile_pool(name="ild_work", bufs=2) as work_pool,
            tc.tile_pool(name="out8", bufs=2) as out8_pool,
            tc.tile_pool(name="band_ps", bufs=5, space=bass.MemorySpace.PSUM) as band_ps_pool,
            tc.tile_pool(name="sums_ps", bufs=2, space=bass.MemorySpace.PSUM) as sums_ps_pool,
        ):
            lr = inp_pool.tile([128, wtot], BF16)
            # input loads go on the ACT HWDGE ring (nc.scalar) so the output
            # DMAs on the sync ring are not stuck behind them (FIFO per
            # ring); chunks interleave L/R so early windows start ASAP
            # the ones column first: every freq-sum matmul reads it, so it
            # must not ride the tail of the bulk load
            nc.scalar.dma_start(lr[:, ones_off:ones_off + 1],
                                packed[:, ones_off:ones_off + 1])
            nsplit = 4
            chunks = []
            for ci in range(nsplit):
                for r0, r1 in ((0, r_off), (r_off, ones_off)):
                    csz = (r1 - r0 + nsplit - 1) // nsplit
                    c0 = r0 + ci * csz
                    c1 = min(r0 + (ci + 1) * csz, r1)
                    chunks.append((c0, c1))
            for c0, c1 in chunks:
                nc.scalar.dma_start(lr[:, c0:c1], packed[:, c0:c1])
            ones = lr[:, ones_off:ones_off + 1]

            evicted = []          # evicted band slices, in order
            staging = {}          # (b, g8) -> staging tile

            for g in range(ngroups):
                sums_ps = [
                    sums_ps_pool.tile([128, 2 * group], F32, tag="sums",
                                      name=f"sums_g{g}b{bb}")
                    for bb in range(B_LOC)
                ]
                wl = 0
                while wl < group:
                    npair = 2 if wl + 1 < group else 1
                    w0 = g * group + wl
                    for b in range(B_LOC):
                        bp = 64 * b
                        ps = band_ps_pool.tile([128, NSTREAM * npair], F32, tag="band")
                        for i in range(npair):
                            tr = (w0 + i) * WIN
                            nc.tensor.matmul(
                                ps[:, i * NSTREAM:(i + 1) * NSTREAM],
                                lr[bp:bp + 64, r_off + tr: r_off + tr + WIN],
                                lr[bp:bp + 64, l_off + tr - PAD: l_off + tr - PAD + NSTREAM],
                            )
                            # freq-sum matmuls for ILD (N=1, ones rhs);
                            # R-sum first: same stationary as the band matmul
                            w_abs = wl + i
                            nc.tensor.matmul(
                                sums_ps[b][:, 2 * w_abs + 1: 2 * w_abs + 2],
                                lr[bp:bp + 64, r_off + tr: r_off + tr + WIN],
                                ones[bp:bp + 64, :],
                            )
                            nc.tensor.matmul(
                                sums_ps[b][:, 2 * w_abs: 2 * w_abs + 1],
                                lr[bp:bp + 64, l_off + tr: l_off + tr + WIN],
                                ones[bp:bp + 64, :],
                            )
                        width = NSTREAM * npair
                        p_idx = w0 // 2
                        g8, pg = p_idx // GB, p_idx % GB
                        if (b, g8) not in staging:
                            staging[(b, g8)] = band_pool.tile(
                                [128, GB * 2 * NSTREAM], BF16, tag="band",
                                name=f"stage_b{b}g{g8}")
                        stage = staging[(b, g8)]
                        band = stage[:, pg * 2 * NSTREAM: pg * 2 * NSTREAM + width]
                        if (p_idx + b) % 2 == 0:
                            nc.vector.tensor_copy(band, ps[:])
                        else:
                            nc.scalar.copy(band, ps[:])
                        evicted.append(band)
                        # group filled (or last pair): ship 8 pairs in one DMA
                        if pg == GB - 1 or p_idx == n_pairs - 1:
                            gw = pg * 2 * NSTREAM + width
                            nc.sync.dma_start(band_out[b, g8, :, :gw],
                                              stage[:, :gw])
                    wl += npair

                # ---- ILD for this group ----
                for b in range(B_LOC):
                    sums_sb = sums_sb_pool.tile([128, 2 * group], F32)
                    nc.vector.tensor_copy(sums_sb[:], sums_ps[b][:])
                    sap = sums_sb[:]
                    ls = bass.AP(sap.tensor, sap.offset,
                                 [[2 * group, 128], [2, group]])
                    rs = bass.AP(sap.tensor, sap.offset + 1,
                                 [[2 * group, 128], [2, group]])
                    d_t = work_pool.tile([128, group], F32, tag="d")
                    nc.vector.tensor_sub(d_t[:], ls, rs)
                    s_t = work_pool.tile([128, group], F32, tag="s")
                    nc.vector.scalar_tensor_tensor(
                        s_t[:], ls, 1e-6, rs,
                        op0=mybir.AluOpType.add, op1=mybir.AluOpType.add,
                    )
                    r_t = work_pool.tile([128, group], F32, tag="r")
                    nc.vector.reciprocal(r_t[:], s_t[:])
                    i_t = work_pool.tile([128, group], F32, tag="i")
                    nc.vector.tensor_mul(i_t[:], d_t[:], r_t[:])
                    o8 = out8_pool.tile([128, group * N_ILD], F32)
                    o8ap = o8[:]
                    for k in range(N_ILD):
                        q_t = work_pool.tile([128, group], F32, tag="q")
                        nc.scalar.activation(
                            q_t[:], i_t[:],
                            mybir.ActivationFunctionType.Square,
                            bias=bias_aps[k], scale=float(1.0 / 0.3),
                        )
                        o_view = bass.AP(
                            o8ap.tensor, o8ap.offset + k,
                            [[group * N_ILD, 128], [N_ILD, group]],
                        )
                        nc.scalar.activation(
                            o_view, q_t[:],
                            mybir.ActivationFunctionType.Exp,
                            bias=0.0, scale=-0.5,
                        )
                    nc.sync.dma_start(ild[b, g], o8[:])
    nc.compile()
    return nc


_CACHE = {}


def _get_nc():
    if "nc" not in _CACHE:
        _CACHE["nc"] = build_nc()
    return _CACHE["nc"]


def _pack_inputs(left, right, t_len=T):
    """-> list of per-core input maps. left/right: [B, T, F] f32."""
    import ml_dtypes
    wtot = PAD + t_len + PAD + t_len + 1
    lt = np.ascontiguousarray(left.transpose(0, 2, 1)).astype(ml_dtypes.bfloat16)
    rt = np.ascontiguousarray(right.transpose(0, 2, 1)).astype(ml_dtypes.bfloat16)
    in_maps = []
    for c in range(N_CORES):
        pk = np.zeros((128, wtot), dtype=ml_dtypes.bfloat16)
        pk[:, wtot - 1] = 1.0
        for i in range(B_LOC):
            bidx = B_LOC * c + i
            rows = slice(64 * i, 64 * i + 64)
            pk[rows, PAD:PAD + t_len] = lt[bidx]
            pk[rows, PAD + t_len + PAD: wtot - 1] = rt[bidx]
        in_maps.append({"packed": pk})
    return in_maps


def _band_to_slab(band_f32, t_len):
    """[n_g8, 128, GB*2*NSTREAM] f32 -> slab [t_len, SLABW]:
    slab[128*(2*(GB*g+pg)+i)+m, j] = band[g, m, (pg*2+i)*NSTREAM + m + j]."""
    n_g8, p128, gw = band_f32.shape
    nw = t_len // WIN
    n_win_slots = n_g8 * gw // NSTREAM
    a4 = band_f32.reshape(n_g8, p128, gw // NSTREAM, NSTREAM)
    s0, s1, s2, s3 = a4.strides
    sv = np.lib.stride_tricks.as_strided(
        a4, shape=(n_g8, gw // NSTREAM, WIN, SLABW), strides=(s0, s2, s1 + s3, s3))
    return sv.reshape(n_win_slots, WIN, SLABW)[:nw].reshape(nw * WIN, SLABW)


def _unpack_outputs(results, t_len=T):
    itd = np.zeros((B, t_len, N_ITD), dtype=np.float32)
    ild = np.zeros((B, t_len, N_ILD), dtype=np.float32)
    for c in range(N_CORES):
        band_c = np.asarray(results[c]["band"]).astype(np.float32)
        ild_c = np.asarray(results[c]["ild"])  # [B_LOC, ngroups, 128, group*8]
        ng = ild_c.shape[1]
        grp = ild_c.shape[3] // N_ILD
        for i in range(B_LOC):
            bidx = B_LOC * c + i
            # [g, m, w, k] -> t = 128*(g*grp + w) + m
            ild[bidx] = (
                ild_c[i]
                .reshape(ng, 128, grp, N_ILD)
                .transpose(0, 2, 1, 3)
                .reshape(t_len, N_ILD)
            )
            slab = _band_to_slab(band_c[i], t_len)   # [T, 65]
            for k, d in enumerate(ITD_DELAYS):
                d = int(d)
                j = MAX_ITD - d
                if d >= 0:
                    itd[bidx, :, k] = slab[:, j]
                else:
                    itd[bidx, -d:, k] = slab[: t_len + d, j]
    return itd, ild


def kernel(left_spikes, right_spikes):
    left = np.asarray(left_spikes, dtype=np.float32)
    right = np.asarray(right_spikes, dtype=np.float32)
    nc = _get_nc()
    in_maps = _pack_inputs(left, right)
    res = run_bass_kernel_spmd(nc, in_maps, list(range(N_CORES)))
    _CACHE["last_result"] = res
    return _unpack_outputs(res.results)


# revision 64
# speedup vs baseline: 1.2412x; 1.0612x over previous
"""Trainium2 Bass kernel for the BiauralProcessor problem.

Strategy (per core; batch data-parallel, 2 batches/core on 8 cores):
  - Host packs inputs transposed (freq on partitions) and bf16:
      packed[0:64]   = [zeros32 | L_b0 | zeros32 | R_b0 | ones]
      packed[64:128] = [zeros32 | L_b1 | zeros32 | R_b1 | ones]
  - ITD: for each 128-time window, a PE band matmul with stationary
    R-window [64,128] and streamed L [64,192] produces the full lag
    band in PSUM; every lag in [-32,32] lives on a diagonal.  After a
    DVE/ACT eviction to SBUF (bf16), one DMA with a fused
    (row_pitch+1) element step extracts a contiguous 65-wide diagonal
    slab [128,65] straight to DRAM.  The host selects the 16 delay
    columns.
  - ILD: per-window N=1 matmuls against a ones column accumulate the
    freq-sums L_s/R_s into PSUM; at the end the DVE computes
    (L_s-R_s)/(L_s+R_s+1e-6) and the ACT engine applies the gaussian
    tuning (Square then Exp) for the 8 preferences.

ISA constraint that shaped the code: every engine instruction carries
at most ONE sync-wait (waits on the same semaphore merge).  Hence:
  - two input DMAs only (L region incl. pads / R region incl. ones),
  - the band SBUF staging area is one persistent buffer (no pool slot
    recycling -> evictions wait only on PE, diag DMAs only on the
    evictor),
  - psum slot reuse is preceded by a dummy 1x1 LDWEIGHTS reading the
    last evicted tile, so the PE observes the evictor's semaphore and
    the reusing matmul keeps a single (PE) wait,
  - activation biases are float immediates (static const tensors).
"""

import numpy as np

import concourse.bass as bass
import concourse.bacc as bacc
import concourse.mybir as mybir
import concourse.tile as tile
from concourse.bass_utils import run_bass_kernel_spmd

# ---- problem constants (hardcoded from the spec) ----
B, T, F = 16, 16000, 64
N_CORES = 8
B_LOC = B // N_CORES            # 2 batches per core
MAX_ITD, N_ITD, N_ILD = 32, 16, 8
ITD_DELAYS = np.round(np.linspace(-MAX_ITD, MAX_ITD, N_ITD)).astype(np.int64)
ILD_PREFS = np.linspace(-1.0, 1.0, N_ILD).astype(np.float32)

PAD = 32                        # zero halo around L for the band stream
WIN = 128                       # output times per window
NSTREAM = 192                   # streamed L columns per window
SLABW = 65                      # extracted diagonals (lags -32..32)
GROUP = 125                     # windows per ILD group (one group: the
                                # sums psum tiles are never recycled)

F32 = mybir.dt.float32
BF16 = mybir.dt.bfloat16


def build_nc(t_len=T, group=GROUP):
    nw = t_len // WIN           # windows per batch
    ngroups = nw // group
    assert nw % group == 0
    wtot = PAD + t_len + PAD + t_len + 1   # trailing ones column
    l_off = PAD
    r_off = PAD + t_len + PAD
    ones_off = wtot - 1

    n_pairs = (nw + 1) // 2     # band tiles per batch (last may be single)

    nc = bacc.Bacc("TRN2", target_bir_lowering=False, debug=False)
    packed = nc.dram_tensor("packed", [128, wtot], BF16, kind="ExternalInput")
    # band pairs are staged in SBUF and shipped 8 pairs per DMA (the SP
    # sequencer costs ~600ns per DMA instruction - keep the count low)
    GB = 16                      # pairs per band DMA
    n_g8 = (n_pairs + GB - 1) // GB
    band_out = nc.dram_tensor(
        "band", [B_LOC, n_g8, 128, GB * 2 * NSTREAM], BF16, kind="ExternalOutput")
    # raw [m, w*8+k] layout (contiguous dump; host rearranges to [t, k])
    ild = nc.dram_tensor(
        "ild", [B_LOC, ngroups, 128, group * N_ILD], F32, kind="ExternalOutput")

    # gaussian-tuning bias constants, initialized in the preamble (before
    # the Tile region, behind an all-engine barrier) so activations using
    # them carry no runtime dependency.
    bias_aps = []
    for k in range(N_ILD):
        t = nc.alloc_sbuf_tensor(f"const-bias-{k}", [128, 1], F32)
        nc.gpsimd.memset(t.ap(), float(-ILD_PREFS[k] / 0.3))
        bias_aps.append(t.ap())
    nc.all_engine_barrier()

    with tile.TileContext(nc) as tc:
        with (
            tc.tile_pool(name="inp", bufs=1) as inp_pool,
            tc.tile_pool(name="band_sb", bufs=3) as band_pool,
            tc.tile_pool(name="sums_sb", bufs=2) as sums_sb_pool,
            tc.tile_pool(name="ild_work", bufs=2) as work_pool,
            tc.tile_pool(name="out8", bufs=2) as out8_pool,
            tc.tile_pool(name="band_ps", bufs=6, space=bass.MemorySpace.PSUM) as band_ps_pool,
            tc.tile_pool(name="sums_ps", bufs=2, space=bass.MemorySpace.PSUM) as sums_ps_pool,
        ):
            lr = inp_pool.tile([128, wtot], BF16)
            # input loads go on the ACT HWDGE ring (nc.scalar) so the output
            # DMAs on the sync ring are not stuck behind them (FIFO per
            # ring); chunks interleave L/R so early windows start ASAP
            # the ones column first: every freq-sum matmul reads it, so it
            # must not ride the tail of the bulk load
            nc.scalar.dma_start(lr[:, ones_off:ones_off + 1],
                                packed[:, ones_off:ones_off + 1])
            nsplit = 4
            chunks = []
            for ci in range(nsplit):
                for r0, r1 in ((0, r_off), (r_off, ones_off)):
                    csz = (r1 - r0 + nsplit - 1) // nsplit
                    c0 = r0 + ci * csz
                    c1 = min(r0 + (ci + 1) * csz, r1)
                    chunks.append((c0, c1))
            for c0, c1 in chunks:
                nc.scalar.dma_start(lr[:, c0:c1], packed[:, c0:c1])
            ones = lr[:, ones_off:ones_off + 1]

            evicted = []          # evicted band slices, in order
            staging = {}          # (b, g8) -> staging tile

            for g in range(ngroups):
                sums_ps = [
                    sums_ps_pool.tile([128, 2 * group], F32, tag="sums",
                                      name=f"sums_g{g}b{bb}")
                    for bb in range(B_LOC)
                ]
                wl = 0
                while wl < group:
                    npair = 2 if wl + 1 < group else 1
                    w0 = g * group + wl
                    for b in range(B_LOC):
                        bp = 64 * b
                        ps = band_ps_pool.tile([128, NSTREAM * npair], F32, tag="band")
                        for i in range(npair):
                            tr = (w0 + i) * WIN
                            nc.tensor.matmul(
                                ps[:, i * NSTREAM:(i + 1) * NSTREAM],
                                lr[bp:bp + 64, r_off + tr: r_off + tr + WIN],
                                lr[bp:bp + 64, l_off + tr - PAD: l_off + tr - PAD + NSTREAM],
                            )
                            # freq-sum matmuls for ILD (N=1, ones rhs);
                            # R-sum first: same stationary as the band matmul
                            w_abs = wl + i
                            nc.tensor.matmul(
                                sums_ps[b][:, 2 * w_abs + 1: 2 * w_abs + 2],
                                lr[bp:bp + 64, r_off + tr: r_off + tr + WIN],
                                ones[bp:bp + 64, :],
                            )
                            nc.tensor.matmul(
                                sums_ps[b][:, 2 * w_abs: 2 * w_abs + 1],
                                lr[bp:bp + 64, l_off + tr: l_off + tr + WIN],
                                ones[bp:bp + 64, :],
                            )
                        width = NSTREAM * npair
                        p_idx = w0 // 2
                        g8, pg = p_idx // GB, p_idx % GB
                        if (b, g8) not in staging:
                            staging[(b, g8)] = band_pool.tile(
                                [128, GB * 2 * NSTREAM], BF16, tag="band",
                                name=f"stage_b{b}g{g8}")
                        stage = staging[(b, g8)]
                        band = stage[:, pg * 2 * NSTREAM: pg * 2 * NSTREAM + width]
                        if (p_idx + b) % 2 == 0:
                            nc.vector.tensor_copy(band, ps[:])
                        else:
                            nc.scalar.copy(band, ps[:])
                        evicted.append(band)
                        # group filled (or last pair): ship 8 pairs in one DMA
                        if pg == GB - 1 or p_idx == n_pairs - 1:
                            gw = pg * 2 * NSTREAM + width
                            nc.sync.dma_start(band_out[b, g8, :, :gw],
                                              stage[:, :gw])
                    wl += npair

                # ---- ILD for this group ----
                for b in range(B_LOC):
                    sums_sb = sums_sb_pool.tile([128, 2 * group], F32)
                    nc.vector.tensor_copy(sums_sb[:], sums_ps[b][:])
                    sap = sums_sb[:]
                    ls = bass.AP(sap.tensor, sap.offset,
                                 [[2 * group, 128], [2, group]])
                    rs = bass.AP(sap.tensor, sap.offset + 1,
                                 [[2 * group, 128], [2, group]])
                    d_t = work_pool.tile([128, group], F32, tag="d")
                    nc.vector.tensor_sub(d_t[:], ls, rs)
                    s_t = work_pool.tile([128, group], F32, tag="s")
                    nc.vector.scalar_tensor_tensor(
                        s_t[:], ls, 1e-6, rs,
                        op0=mybir.AluOpType.add, op1=mybir.AluOpType.add,
                    )
                    r_t = work_pool.tile([128, group], F32, tag="r")
                    nc.vector.reciprocal(r_t[:], s_t[:])
                    i_t = work_pool.tile([128, group], F32, tag="i")
                    nc.vector.tensor_mul(i_t[:], d_t[:], r_t[:])
                    o8 = out8_pool.tile([128, group * N_ILD], F32)
                    o8ap = o8[:]
                    for k in range(N_ILD):
                        q_t = work_pool.tile([128, group], F32, tag="q")
                        nc.scalar.activation(
                            q_t[:], i_t[:],
                            mybir.ActivationFunctionType.Square,
                            bias=bias_aps[k], scale=float(1.0 / 0.3),
                        )
                        o_view = bass.AP(
                            o8ap.tensor, o8ap.offset + k,
                            [[group * N_ILD, 128], [N_ILD, group]],
                        )
                        nc.scalar.activation(
                            o_view, q_t[:],
                            mybir.ActivationFunctionType.Exp,
                            bias=0.0, scale=-0.5,
                        )
                    nc.sync.dma_start(ild[b, g], o8[:])
    nc.compile()
    return nc


_CACHE = {}


def _get_nc():
    if "nc" not in _CACHE:
        _CACHE["nc"] = build_nc()
    return _CACHE["nc"]


def _pack_inputs(left, right, t_len=T):
    """-> list of per-core input maps. left/right: [B, T, F] f32."""
    import ml_dtypes
    wtot = PAD + t_len + PAD + t_len + 1
    lt = np.ascontiguousarray(left.transpose(0, 2, 1)).astype(ml_dtypes.bfloat16)
    rt = np.ascontiguousarray(right.transpose(0, 2, 1)).astype(ml_dtypes.bfloat16)
    in_maps = []
    for c in range(N_CORES):
        pk = np.zeros((128, wtot), dtype=ml_dtypes.bfloat16)
        pk[:, wtot - 1] = 1.0
        for i in range(B_LOC):
            bidx = B_LOC * c + i
            rows = slice(64 * i, 64 * i + 64)
            pk[rows, PAD:PAD + t_len] = lt[bidx]
            pk[rows, PAD + t_len + PAD: wtot - 1] = rt[bidx]
        in_maps.append({"packed": pk})
    return in_maps


def _band_to_slab(band_f32, t_len):
    """[n_g8, 128, GB*2*NSTREAM] f32 -> slab [t_len, SLABW]:
    slab[128*(2*(GB*g+pg)+i)+m, j] = band[g, m, (pg*2+i)*NSTREAM + m + j]."""
    n_g8, p128, gw = band_f32.shape
    nw = t_len // WIN
    n_win_slots = n_g8 * gw // NSTREAM
    a4 = band_f32.reshape(n_g8, p128, gw // NSTREAM, NSTREAM)
    s0, s1, s2, s3 = a4.strides
    sv = np.lib.stride_tricks.as_strided(
        a4, shape=(n_g8, gw // NSTREAM, WIN, SLABW), strides=(s0, s2, s1 + s3, s3))
    return sv.reshape(n_win_slots, WIN, SLABW)[:nw].reshape(nw * WIN, SLABW)


def _unpack_outputs(results, t_len=T):
    itd = np.zeros((B, t_len, N_ITD), dtype=np.float32)
    ild = np.zeros((B, t_len, N_ILD), dtype=np.float32)
    for c in range(N_CORES):
        band_c = np.asarray(results[c]["band"]).astype(np.float32)
        ild_c = np.asarray(results[c]["ild"])  # [B_LOC, ngroups, 128, group*8]
        ng = ild_c.shape[1]
        grp = ild_c.shape[3] // N_ILD
        for i in range(B_LOC):
            bidx = B_LOC * c + i
            # [g, m, w, k] -> t = 128*(g*grp + w) + m
            ild[bidx] = (
                ild_c[i]
                .reshape(ng, 128, grp, N_ILD)
                .transpose(0, 2, 1, 3)
                .reshape(t_len, N_ILD)
            )
            slab = _band_to_slab(band_c[i], t_len)   # [T, 65]
            for k, d in enumerate(ITD_DELAYS):
                d = int(d)
                j = MAX_ITD - d
                if d >= 0:
                    itd[bidx, :, k] = slab[:, j]
                else:
                    itd[bidx, -d:, k] = slab[: t_len + d, j]
    return itd, ild


def kernel(left_spikes, right_spikes):
    left = np.asarray(left_spikes, dtype=np.float32)
    right = np.asarray(right_spikes, dtype=np.float32)
    nc = _get_nc()
    in_maps = _pack_inputs(left, right)
    res = run_bass_kernel_spmd(nc, in_maps, list(range(N_CORES)))
    _CACHE["last_result"] = res
    return _unpack_outputs(res.results)
